# revision 12
# baseline (speedup 1.0000x reference)
"""Trainium2 Bass kernel for nn_BagModel_3d (segment_reduce).

Computation (per bag b):
  out[b] = (1/n_b) * sum_{i < n_b} relu(x[b, i, :] @ W1 + b1) @ W2 + b2

Strategy: data-parallel over bags across 8 cores, with the Bass program
JIT-specialized on the n_instances vector (the same way b2 is baked in):

- Only the valid instances of each bag are transferred and computed
  (n_b rounded up to 16; pad columns zeroed). Bags are assigned to cores
  by greedy LPT on padded size so every core moves about the same bytes;
  outputs are permuted back on host. Because all 8 cores run one SPMD
  program, slot s is sized max-over-cores (cores' slot sequences are
  sorted descending, so the overhead is a few %).
- Slots are packed into GROUPS of <= 512 instance columns. Per group:
  one DMA ([128, 2W] f32 from a contiguous group-major DRAM block,
  sync/scalar HWDGE rings alternating by running byte balance), one
  f32->bf16 cast (VectorE 2x-mode / GpSimd, alternating), and 4 bf16
  matmuls (2 dh-chunks x 2 k-halves) into 2 PSUM banks. bf16 keeps the
  PE at 1 row/cycle and lets the HAM clock-gate ramp to 2.4 GHz, unlike
  fp32/fp32r whose streaming is SBUF-bandwidth-limited.
- Per bag, each PSUM column range is drained once with a fused
  relu(z + b1) + free-axis accumulation into praw[:, slot]: dh chunk 0
  on ScalarE (activation), chunk 1 on VectorE (scalar_tensor_tensor).
  Zeroed pad columns contribute relu(b1) each; a rank-1 (n_b - np_b) x
  relu(b1) correction (exactly 0 for the spec's b1=0) restores the true
  sum. The mean's 1/n and +b2 fold into one per-partition op on the
  final [bags,1] PSUM output of the W2 matmul.
"""
import sys
import numpy as np

sys.path.insert(0, '/opt/trn_rl_repo')

B, N_MAX, D_IN, D_H = 256, 512, 256, 256
N_CORES = 8
BAGS = B // N_CORES          # 32 bags per core
GROUP_W = 512                # max instance columns per compute group
PF = 8                       # group-DMA prefetch depth
ALIGN = 16                   # pad n to multiple of 16 (64B f32 rows)

_PROGRAM = None
_PROGRAM_KEY = None
_PLAN = None


def _make_plan(n, b2_value):
    n = np.asarray(n, dtype=np.int64)
    npad = ((n + ALIGN - 1) // ALIGN) * ALIGN
    order = np.argsort(-npad, kind="stable")
    loads = [0] * N_CORES
    assign = [[] for _ in range(N_CORES)]
    for b in order:
        cands = [i for i in range(N_CORES) if len(assign[i]) < BAGS]
        c = min(cands, key=lambda i: (loads[i], len(assign[i]), i))
        assign[c].append(int(b))
        loads[c] += int(npad[b])
    # SPMD slot widths: max over cores per slot (cores are sorted desc)
    slot_w = [max(int(npad[assign[c][s]]) for c in range(N_CORES))
              for s in range(BAGS)]
    # pack slots into groups of <= GROUP_W columns
    groups = []           # list of (slot_start, slot_end, [widths], W)
    s = 0
    while s < BAGS:
        e, tot = s, 0
        while e < BAGS and tot + slot_w[e] <= GROUP_W:
            tot += slot_w[e]
            e += 1
        groups.append((s, e, slot_w[s:e], tot))
        s = e
    return {
        "assign": assign,
        "slot_w": slot_w,
        "groups": groups,
        "n": [int(v) for v in n],
        "b2": float(b2_value),
    }


def _build_program(plan):
    import concourse.bacc as bacc
    import concourse.tile as tile
    from concourse import mybir

    f32 = mybir.dt.float32
    bf16 = mybir.dt.bfloat16
    i32 = mybir.dt.int32
    Alu = mybir.AluOpType
    Act = mybir.ActivationFunctionType

    slot_w = plan["slot_w"]
    groups = plan["groups"]
    TOT2 = 2 * sum(g[3] for g in groups)
    goff = []             # column offset of each group in xall
    off = 0
    for (_, _, _, wg) in groups:
        goff.append(off)
        off += 2 * wg
    assert off == TOT2

    nc = bacc.Bacc("TRN2", target_bir_lowering=False, debug=False)

    xall = nc.dram_tensor("xall", [128, TOT2], f32, kind="ExternalInput").ap()
    n_col = nc.dram_tensor("n_col", [BAGS, 1], i32, kind="ExternalInput").ap()
    n_row = nc.dram_tensor("n_row", [1, BAGS], i32, kind="ExternalInput").ap()
    npad_row = nc.dram_tensor("npad_row", [1, BAGS], f32, kind="ExternalInput").ap()
    w1 = nc.dram_tensor("w1", [D_IN, D_H], f32, kind="ExternalInput").ap()
    b1 = nc.dram_tensor("b1", [D_H, 1], f32, kind="ExternalInput").ap()
    w2 = nc.dram_tensor("w2", [D_H, 1], f32, kind="ExternalInput").ap()
    out = nc.dram_tensor("out", [BAGS, 1], f32, kind="ExternalOutput").ap()

    with tile.TileContext(nc) as tc:
        with (
            tc.tile_pool(name="const", bufs=1) as cpool,
            tc.tile_pool(name="xb", bufs=PF + 2) as xbfpool,
            tc.tile_pool(name="xf", bufs=6) as xpool,
            tc.tile_pool(name="h", bufs=4) as hpool,
            tc.tile_pool(name="z", bufs=6, space="PSUM") as zpool,
            tc.tile_pool(name="smallps", bufs=1, space="PSUM") as spspool,
        ):
            # ---- weights first (first matmul needs them) ----
            w1k0 = cpool.tile([128, D_H], f32, tag="w1k0")
            w1k1 = cpool.tile([128, D_H], f32, tag="w1k1")
            nc.sync.dma_start(w1k0[:], w1[0:128, :])
            nc.scalar.dma_start(w1k1[:], w1[128:256, :])
            w1b0 = cpool.tile([128, D_H], bf16, tag="w1b0")
            nc.scalar.copy(w1b0[:], w1k0[:])
            w1b1 = cpool.tile([128, D_H], bf16, tag="w1b1")
            nc.vector.tensor_copy(w1b1[:], w1k1[:])

            # small tensors on the sync HWDGE ring (gpsimd/SWDGE carries x,
            # and anything queued ahead of x on the Q7 delays the stream)
            b1t = cpool.tile([128, 2], f32, tag="b1t")
            nc.sync.dma_start(b1t[:, 0:1], b1[0:128, :])
            nc.sync.dma_start(b1t[:, 1:2], b1[128:256, :])
            w2t = cpool.tile([128, 2], f32, tag="w2t")
            nc.sync.dma_start(w2t[:, 0:1], w2[0:128, :])
            nc.sync.dma_start(w2t[:, 1:2], w2[128:256, :])
            nI_col = cpool.tile([BAGS, 1], i32, tag="nI_col")
            nc.sync.dma_start(nI_col[:], n_col[:])
            nI_row = cpool.tile([1, BAGS], i32, tag="nI_row")
            nc.sync.dma_start(nI_row[:], n_row[:])
            npadf_row = cpool.tile([1, BAGS], f32, tag="npadf_row")
            nc.scalar.dma_start(npadf_row[:], npad_row[:])
            b1row = cpool.tile([1, D_H], f32, tag="b1row")
            nc.scalar.dma_start(b1row[:], b1.transpose([1, 0]))

            # ---- n-derived scalars (vector, during DMA fill) ----
            nf_col = cpool.tile([BAGS, 1], f32, tag="nf_col")
            nc.vector.tensor_copy(nf_col[:], nI_col[:])
            inv_col = cpool.tile([BAGS, 1], f32, tag="inv_col")
            nc.vector.reciprocal(inv_col[:], nf_col[:])
            nf_row = cpool.tile([1, BAGS], f32, tag="nf_row")
            nc.vector.tensor_copy(nf_row[:], nI_row[:])
            cnt_row = cpool.tile([1, BAGS], f32, tag="cnt_row")
            nc.vector.tensor_tensor(cnt_row[:], nf_row[:], npadf_row[:],
                                    op=Alu.subtract)
            rb1row = cpool.tile([1, D_H], f32, tag="rb1row")
            nc.vector.tensor_scalar(rb1row[:], b1row[:], 0.0, None, op0=Alu.max)

            praw0 = cpool.tile([128, BAGS], f32, tag="praw0")
            praw1 = cpool.tile([128, BAGS], f32, tag="praw1")
            zeros_t = cpool.tile([128, N_MAX], f32, tag="zeros_t")
            nc.vector.memset(zeros_t[:], 0.0)

            # ---- group stream + pipeline ----
            # SWDGE (gpsimd) loads bf16 via inline cast but its Q7 takes
            # ~6-7us to start issuing; the first KH groups go on the HWDGE
            # rings as f32 (ready immediately) and VectorE casts them while
            # it is still otherwise idle.
            NG = len(groups)
            KH = min(5, NG)
            xb_t = [None] * NG
            xf_t = [None] * NG

            def issue_x(g):
                wg = groups[g][3]
                if g < KH:
                    xf = xpool.tile([128, 2 * wg], f32, tag="xf", name=f"xf_{g}",
                                    padded_shape=[128, 2 * GROUP_W])
                    (nc.sync, nc.scalar)[g % 2].dma_start(
                        xf[:], xall[:, goff[g]:goff[g] + 2 * wg])
                    xf_t[g] = xf
                else:
                    xb = xbfpool.tile([128, 2 * wg], bf16, tag="xb",
                                      name=f"xb_{g}",
                                      padded_shape=[128, 2 * GROUP_W])
                    nc.gpsimd.dma_start(xb[:], xall[:, goff[g]:goff[g] + 2 * wg])
                    xb_t[g] = xb

            for g in range(min(PF, NG)):
                issue_x(g)

            drain_flip = 0
            for g in range(NG):
                if g + PF < NG:
                    issue_x(g + PF)
                s0, s1, widths, wg = groups[g]
                if g < KH:
                    xb = xbfpool.tile([128, 2 * wg], bf16, tag="xb",
                                      name=f"xbc_{g}",
                                      padded_shape=[128, 2 * GROUP_W])
                    nc.vector.tensor_copy(xb[:], xf_t[g][:])
                else:
                    xb = xb_t[g]
                x0 = xb[:, 0:wg]
                x1 = xb[:, wg:2 * wg]
                z0 = zpool.tile([128, wg], f32, tag="z", name=f"z0_{g}",
                                padded_shape=[128, GROUP_W])
                nc.tensor.matmul(z0[:], w1b0[:, 0:128], x0, start=True, stop=False)
                nc.tensor.matmul(z0[:], w1b1[:, 0:128], x1, start=False, stop=True)
                z1 = zpool.tile([128, wg], f32, tag="z", name=f"z1_{g}",
                                padded_shape=[128, GROUP_W])
                nc.tensor.matmul(z1[:], w1b0[:, 128:256], x0, start=True, stop=False)
                nc.tensor.matmul(z1[:], w1b1[:, 128:256], x1, start=False, stop=True)
                # per-bag fused relu(z + b1) + row-sum drains, alternating
                # engines; engine e handles chunk e's PSUM for half the
                # slots and chunk (1-e)'s for the other half
                loff = 0
                for s in range(s0, s1):
                    w = slot_w[s]
                    za, zb_ = (z0, z1) if drain_flip == 0 else (z1, z0)
                    pa, pb = (praw0, praw1) if drain_flip == 0 else (praw1, praw0)
                    ba, bb = (0, 1) if drain_flip == 0 else (1, 0)
                    h0 = hpool.tile([128, w], f32, tag="hs", name=f"h0_{s}",
                                    padded_shape=[128, GROUP_W])
                    nc.scalar.activation(h0[:], za[:, loff:loff + w], Act.Relu,
                                         bias=b1t[:, ba:ba + 1], scale=1.0,
                                         accum_out=pa[:, s:s + 1])
                    h1 = hpool.tile([128, w], f32, tag="hv", name=f"h1_{s}",
                                    padded_shape=[128, GROUP_W])
                    nc.vector.scalar_tensor_tensor(
                        h1[:], zb_[:, loff:loff + w], b1t[:, bb:bb + 1],
                        zeros_t[:, 0:w],
                        op0=Alu.add, op1=Alu.max,
                        accum_out=pb[:, s:s + 1])
                    drain_flip ^= 1
                    loff += w

            # ---- padding correction + final Linear ----
            pscs = []
            for c in range(2):
                corr = spspool.tile([128, BAGS], f32, tag="corr", name=f"corr{c}")
                nc.tensor.matmul(corr[:], rb1row[0:1, 128 * c:128 * (c + 1)],
                                 cnt_row[:], start=True, stop=True)
                psc = cpool.tile([128, BAGS], f32, tag=f"psc{c}")
                nc.vector.tensor_add(psc[:], (praw0, praw1)[c][:], corr[:])
                pscs.append(psc)
            po = spspool.tile([BAGS, 1], f32, tag="po")
            nc.tensor.matmul(po[:], pscs[0][:], w2t[:, 0:1], start=True, stop=False)
            nc.tensor.matmul(po[:], pscs[1][:], w2t[:, 1:2], start=False, stop=True)
            osb = cpool.tile([BAGS, 1], f32, tag="osb")
            nc.vector.tensor_scalar(
                osb[:], po[:], inv_col[:, 0:1], float(plan["b2"]),
                op0=Alu.mult, op1=Alu.add)
            nc.sync.dma_start(out[:], osb[:])

    nc.compile()
    return nc


def get_program(plan):
    global _PROGRAM, _PROGRAM_KEY
    key = (plan["b2"], tuple(plan["n"]))
    if _PROGRAM is None or _PROGRAM_KEY != key:
        _PROGRAM = _build_program(plan)
        _PROGRAM_KEY = key
    return _PROGRAM


def make_in_maps(x, n_instances, W1, b1, W2, b2=None):
    global _PLAN
    x = np.asarray(x, dtype=np.float32)
    n = np.asarray(n_instances, dtype=np.int32)
    W1 = np.asarray(W1, dtype=np.float32)
    b1 = np.asarray(b1, dtype=np.float32).reshape(D_H, 1)
    W2 = np.asarray(W2, dtype=np.float32).reshape(D_H, 1)
    b2v = 0.0 if b2 is None else float(np.asarray(b2).reshape(-1)[0])
    plan = _make_plan(n, b2v)
    _PLAN = plan
    assign, slot_w, groups = plan["assign"], plan["slot_w"], plan["groups"]
    TOT2 = 2 * sum(g[3] for g in groups)
    in_maps = []
    for c in range(N_CORES):
        bags = assign[c]
        im = {"w1": W1, "b1": b1, "w2": W2}
        ns = np.array([n[b] for b in bags], dtype=np.int32)
        im["n_col"] = np.ascontiguousarray(ns.reshape(BAGS, 1))
        im["n_row"] = np.ascontiguousarray(ns.reshape(1, BAGS))
        im["npad_row"] = np.array([[float(w) for w in slot_w]], dtype=np.float32)
        xa = np.zeros((128, TOT2), dtype=np.float32)
        off = 0
        for (s0, s1, widths, wg) in groups:
            loff = 0
            for s in range(s0, s1):
                w = slot_w[s]
                nb = int(n[bags[s]])
                xv = x[bags[s], :nb, :].T          # [256, nb]
                xa[:, off + loff:off + loff + nb] = xv[0:128, :]
                xa[:, off + wg + loff:off + wg + loff + nb] = xv[128:256, :]
                loff += w
            off += 2 * wg
        im["xall"] = xa
        in_maps.append(im)
    return in_maps


def run_spmd(in_maps, b2_value=0.0, trace=False, **kwargs):
    from concourse import bass_utils
    if trace:
        # no S3 in this environment; keep trace artifacts local
        bass_utils.upload_artifacts = lambda tmpdir: tmpdir
    nc = get_program(_PLAN)
    return bass_utils.run_bass_kernel_spmd(
        nc, in_maps, core_ids=list(range(N_CORES)), trace=trace, **kwargs)


def kernel(x, n_instances, W1, b1, W2, b2):
    b2_value = float(np.asarray(b2).reshape(-1)[0])
    in_maps = make_in_maps(x, n_instances, W1, b1, W2, b2)
    res = run_spmd(in_maps, b2_value=b2_value)
    out = np.empty((B, 1), dtype=np.float32)
    for c in range(N_CORES):
        vals = res.results[c]["out"]
        for s, b in enumerate(_PLAN["assign"][c]):
            out[b, 0] = vals[s, 0]
    return out
